# revision 6
# baseline (speedup 1.0000x reference)
"""Trainium2 Bass kernel for nn_AlignmentModule (conv stems + L2 score +
log-softmax + beta-binomial prior).

Sharding: 8 cores = 4 batches x 2 T_feats halves (400 rows each). Each core
computes the text conv stem for its batch, its half of the feats conv stem,
and the (400, 160) score block in FEATS-MAJOR layout (feats positions on
psum partitions in 128/128/128/16 chunks, text on the free dim).

Feats-major design (vs the text-major ~31us baseline):
 - d2 = |f|^2 - 2 f.t + |t|^2 assembled per chunk in PSUM:
   ones^T @ t^2 matmuls give |t_j|^2 (start), fneg2^T @ t give -2 f.t,
   and a K=1 rank-1 (|f|^2 row x ones) closes the group. |f|^2 row comes
   from one ones-column matmul over ffn = (-2f)^2 (scaled 0.25 on evac).
 - softmax over text = the FREE dim: the ACT Exp pass's accum_out port
   yields Z per feats row for free; out = (prior - s) - lnZ is two cheap
   DVE passes. No transposes anywhere, no PE work in the softmax, and the
   output leaves in the reference (feats, text) orientation as fp16.
 - PE warm-up: 8 N=512 dummy matmuls issued immediately (~3.4us at the
   cold 1.2 GHz clock) so the free-running HAM window unthrottles the PE
   to 2.4 GHz right as the input DMAs land and the real matmuls start.
 - input DMA descriptors spread across 4 engine queues (sync/scalar/
   gpsimd/vector) instead of serializing on one.
 - all ACT functions used (exp/ln/relu/identity/square) live in the
   natural_log_exp_and_others table set -> one table load, warmed at t=0.

Self-contained: hardcodes all shapes; reads nothing from disk.
"""

import math
import os
import subprocess
import sys

import numpy as np
import ml_dtypes

import concourse.bass as bass
import concourse.mybir as mybir
import concourse.tile as tile
from concourse.bass_utils import run_bass_kernel_spmd

B, T_TEXT, T_FEATS = 4, 160, 800
ADIM, ODIM = 256, 80
N_CORES = 8
HALF = T_FEATS // 2          # 400 feats rows per core
TT = T_TEXT                  # 160
TFW = HALF + 2               # 402: f1 window  [s-1, s+401)
TFIN = HALF + 4              # 404: feats input window [s-2, s+402)
TTP = TT + 2                 # 162: text input window (zero halo cols)
F32 = mybir.dt.float32
BF16 = mybir.dt.bfloat16
FP16 = mybir.dt.float16
MASK_PENALTY = 6.0e4         # d2 += 6e4 -> s ~ 245 -> exp(-s) == 0 in f32
CHUNKS = [(0, 128), (128, 128), (256, 128), (384, 16)]

# ---- input pack layouts -------------------------------------------------
# pk_fw (80, PKF_W) bf16: featsT window | fw1
PKF_FEATS = 0
PKF_FW1 = TFIN
PKF_W = TFIN + 3 * 256

# pk_a1 (128, PKA1_W) bf16: textT (2ci x TTP) | tw1 (2ci x 768) | f1 halo masks
PKA1_TEXT = 0
PKA1_TW1 = 2 * TTP
PKA1_MASK = PKA1_TW1 + 2 * 768
PKA1_W = PKA1_MASK + 2

# pk_a2 (128, 1536) bf16: fw2
PKA2_W = 2 * 768

# pk_a3 (128, 1024) bf16: tw2 (2ci x 256) | fw3 (2ci x 256)
PKA3_TW2 = 0
PKA3_FW3 = 2 * 256
PKA3_W = 4 * 256

# pk_p (128, PKP_W) fp16: prior chunks (4 x 160, chunk c rows at cols
# [160c,160c+160)) | mask penalty row (row 0 of cols 640:800)
PKP_PRIOR = 0
PKP_MASK = 4 * TT
PKP_W = 5 * TT

# pk_b (128, 12) f32 bias columns (2 co-halves each)
BI_FB1 = 0    # f_b1
BI_FB2 = 2    # f_b2
BI_TB1 = 4    # t_b1
BI_TB2 = 6    # t_b2
BI_FB3P = 8   # +f_b3
BI_FB3N = 10  # -2*f_b3
PKB_W = 12

_nc_cache = {}
_prior_cache = None


# ---------------------------------------------------------------- host math
def _prior_f64():
    """f64 fallback replica of reference.beta_binomial_prior."""
    try:
        from scipy.special import gammaln as _gl
    except Exception:
        _gl = np.vectorize(math.lgamma)
    T, N = float(T_FEATS), float(T_TEXT)
    a = np.arange(1, T_FEATS + 1, dtype=np.float64)
    b = T - a + 1.0
    k = np.arange(T_TEXT, dtype=np.float64)[:, None]

    def betaln(x, y):
        return _gl(x) + _gl(y) - _gl(x + y)

    logp = (
        _gl(N + 1.0) - _gl(k + 1.0) - _gl(N - k + 1.0)
        + betaln(k + a, N - k + b) - betaln(a, b)
    )
    return np.asarray(logp.T, dtype=np.float32)


_NIX_SITE = ("/nix/store/z022hj2nvbm3nwdizlisq4ylc0y7rd6q-python3-3.13.14-env"
             "/lib/python3.13/site-packages")

_PRIOR_SRC = """
import os
os.environ["JAX_PLATFORMS"] = "cpu"
import numpy as np
import jax.numpy as jnp
from jax.scipy.special import gammaln

T, N = {T}, {N}
a = 1.0 * jnp.arange(1, T + 1, dtype=jnp.float32)
b = 1.0 * (T - a + 1.0)
k = jnp.arange(N, dtype=jnp.float32)[:, None]
Nf = jnp.float32(N)

def betaln(x, y):
    return gammaln(x) + gammaln(y) - gammaln(x + y)

logp = (gammaln(Nf + 1.0) - gammaln(k + 1.0) - gammaln(Nf - k + 1.0)
        + betaln(k + a, Nf - k + b) - betaln(a, b))
np.save({out!r}, np.asarray(logp.T, dtype=np.float32))
"""


def _beta_binomial_prior():
    """beta_binomial_prior(T_FEATS, T_TEXT) matching the reference's jax
    f32 computation, via a jax-CPU subprocess (python -S skips the
    sitecustomize that would force the axon/neuron backend). Falls back
    to a scipy f64 replica (abs diff ~1e-3, harmless at the 2e-2 gate)."""
    global _prior_cache
    if _prior_cache is not None:
        return _prior_cache
    cache = f"/tmp/_bbprior_{T_FEATS}x{T_TEXT}.npy"
    if not os.path.exists(cache):
        src = _PRIOR_SRC.format(T=T_FEATS, N=T_TEXT, out=cache)
        for _attempt in range(2):
            try:
                env = dict(os.environ)
                env["JAX_PLATFORMS"] = "cpu"
                env["TRN_TERMINAL_POOL_IPS"] = ""
                env["PYTHONPATH"] = _NIX_SITE
                r = subprocess.run([sys.executable, "-S", "-c", src],
                                   capture_output=True, timeout=120, env=env)
                if r.returncode == 0 and os.path.exists(cache):
                    break
            except Exception:
                pass
    if os.path.exists(cache):
        _prior_cache = np.load(cache).astype(np.float32)
    else:
        _prior_cache = _prior_f64()
    return _prior_cache


# ------------------------------------------------------------- BIR patching
def _split_multiwait(nc):
    """This container's walrus accepts at most one sync wait per
    instruction; move extras onto single-wait NOPs just before."""
    for f in nc.m.functions:
        for bb in f.blocks:
            changed = False
            out = []
            for inst in bb.instructions:
                si = inst.sync_info
                if si is not None and len(si.on_wait) > 1:
                    waits = list(si.on_wait)
                    for j, w in enumerate(waits[:-1]):
                        nop = mybir.InstNoOp(name=f"{inst.name}sw{j}")
                        nop.name = f"{inst.name}sw{j}"
                        nop.engine = inst.engine
                        nop.sync_info = mybir.SyncInfo(on_wait=[w], on_update=[])
                        out.append(nop)
                    inst.sync_info = mybir.SyncInfo(
                        on_wait=[waits[-1]], on_update=list(si.on_update)
                    )
                    changed = True
                out.append(inst)
            if changed:
                bb.instructions = out


# ------------------------------------------------------------ device program
def _build_program(with_biases=False):
    if with_biases in _nc_cache:
        return _nc_cache[with_biases]

    nc = bass.Bass("TRN2", target_bir_lowering=False, debug=False,
                   num_devices=N_CORES, enable_asserts=False)
    AF = mybir.ActivationFunctionType
    AL = mybir.AluOpType

    if with_biases:
        d_pb = nc.dram_tensor("pk_b", [128, PKB_W], F32, kind="ExternalInput")
    d_pf = nc.dram_tensor("pk_fw", [ODIM, PKF_W], BF16, kind="ExternalInput")
    d_a1 = nc.dram_tensor("pk_a1", [128, PKA1_W], BF16, kind="ExternalInput")
    d_a2 = nc.dram_tensor("pk_a2", [128, PKA2_W], BF16, kind="ExternalInput")
    d_a3 = nc.dram_tensor("pk_a3", [128, PKA3_W], BF16, kind="ExternalInput")
    d_pp = nc.dram_tensor("pk_p", [128, PKP_W], FP16, kind="ExternalInput")
    d_out = nc.dram_tensor("out", [HALF, TT], FP16, kind="ExternalOutput")

    with tile.TileContext(nc) as tc:
        with (
            tc.tile_pool(name="dpool", bufs=1) as dpool,
            tc.tile_pool(name="spool", bufs=1) as spool,
            tc.tile_pool(name="pwp", bufs=1, space="PSUM") as pwp,
            tc.tile_pool(name="pconv", bufs=3, space="PSUM") as pconv,
            tc.tile_pool(name="pd2", bufs=2, space="PSUM") as pd2p,
            tc.tile_pool(name="psm", bufs=1, space="PSUM") as psm,
        ):
            # ---------------- input DMAs on 4 queues --------------------
            pf = dpool.tile([ODIM, PKF_W], BF16, name="pf")
            nc.sync.dma_start(pf[0:16, :], d_pf.ap()[0:16, :])
            nc.sync.dma_start(pf[16:ODIM, :], d_pf.ap()[16:ODIM, :])
            a1 = dpool.tile([128, PKA1_W], BF16, name="a1")
            nc.scalar.dma_start(a1[:], d_a1.ap())
            a3 = dpool.tile([128, PKA3_W], BF16, name="a3")
            nc.gpsimd.dma_start(a3[:], d_a3.ap())
            a2 = dpool.tile([128, PKA2_W], BF16, name="a2")
            nc.gpsimd.dma_start(a2[:], d_a2.ap())
            pp = dpool.tile([128, PKP_W], FP16, name="pp")
            nc.gpsimd.dma_start(pp[:], d_pp.ap())

            # warm-up source memset first on DVE
            wscr = spool.tile([128, 512], BF16, name="wscr")
            nc.vector.memset(wscr[:], 1.0)

            pb = dpool.tile([128, PKB_W], F32, name="pb")
            if with_biases:
                nc.scalar.dma_start(pb[:], d_pb.ap())
            else:
                nc.gpsimd.memset(pb[:], 0.0)

            def bias(base, j):
                return pb[:, base + j: base + j + 1]

            # ---------------- on-chip constants + warms -----------------
            ones = spool.tile([128, 128], BF16, name="ones")
            nc.gpsimd.memset(ones[:], 1.0)
            onesr = spool.tile([1, TT], FP16, name="onesr")
            nc.gpsimd.memset(onesr[:], 1.0)

            # warm the ACT table set (natural_log_exp_and_others covers
            # exp/ln/relu/identity/square) during the input DMAs
            scr = spool.tile([1, 4], F32, name="scr")
            nc.scalar.activation(scr[0:1, 1:2], scr[0:1, 3:4], AF.Exp)
            nc.scalar.activation(scr[0:1, 2:3], scr[0:1, 3:4], AF.Ln)

            # PE HAM warm-up: ~3.4us of dense cold matmuls so the HAM
            # SHORT window unthrottles the PE right as real work starts
            pwarm = pwp.tile([128, 512], F32, name="pwarm", tag="warm")
            for _ in range(8):
                nc.tensor.matmul(pwarm[:], wscr[:, 0:128], wscr[:],
                                 start=True, stop=True,
                                 skip_group_check=True)

            # ---------------- feats conv1 -------------------------------
            f1_sb = []
            for co in range(2):
                p = pconv.tile([128, TFW], F32, name=f"pf1_{co}", tag="convp")
                for k in range(3):
                    nc.tensor.matmul(
                        p[:],
                        pf[:, PKF_FW1 + 256 * k + 128 * co:
                           PKF_FW1 + 256 * k + 128 * (co + 1)],
                        pf[:, k:k + TFW],
                        start=(k == 0), stop=(k == 2),
                    )
                f1 = spool.tile([128, TFW], BF16, name=f"f1_{co}")
                if co == 0:
                    nc.scalar.activation(f1[:], p[:], AF.Relu,
                                         bias=bias(BI_FB1, co))
                else:
                    nc.vector.tensor_scalar(f1[:], p[:], bias(BI_FB1, co), 0.0,
                                            op0=AL.add, op1=AL.max)
                # zero the halo column the reference conv padding zeroes
                nc.vector.tensor_mul(f1[:, 0:1], f1[:, 0:1],
                                     a1[:, PKA1_MASK:PKA1_MASK + 1])
                nc.vector.tensor_mul(f1[:, TFW - 1:TFW], f1[:, TFW - 1:TFW],
                                     a1[:, PKA1_MASK + 1:PKA1_MASK + 2])
                f1_sb.append(f1)

            # ---------------- text conv1 --------------------------------
            t1_sb = []
            for co in range(2):
                p = pconv.tile([128, TT], F32, name=f"pt1_{co}", tag="convp",
                               padded_shape=[128, TFW])
                n = 0
                for ci in range(2):
                    for k in range(3):
                        nc.tensor.matmul(
                            p[:],
                            a1[:, PKA1_TW1 + 768 * ci + 256 * k + 128 * co:
                               PKA1_TW1 + 768 * ci + 256 * k + 128 * (co + 1)],
                            a1[:, TTP * ci + k: TTP * ci + k + TT],
                            start=(n == 0), stop=(n == 5),
                        )
                        n += 1
                t1 = spool.tile([128, TT], BF16, name=f"t1_{co}")
                if co == 0:
                    nc.scalar.activation(t1[:], p[:], AF.Relu,
                                         bias=bias(BI_TB1, co))
                else:
                    nc.vector.tensor_scalar(t1[:], p[:], bias(BI_TB1, co), 0.0,
                                            op0=AL.add, op1=AL.max)
                t1_sb.append(t1)

            # ---------------- text conv2: t, tt = t^2 -------------------
            t_sb, tt_sb = [], []
            for co in range(2):
                p = pconv.tile([128, TT], F32, name=f"pt2_{co}", tag="convp",
                               padded_shape=[128, TFW])
                for ci in range(2):
                    nc.tensor.matmul(
                        p[:],
                        a3[:, PKA3_TW2 + 256 * ci + 128 * co:
                           PKA3_TW2 + 256 * ci + 128 * (co + 1)],
                        t1_sb[ci][:],
                        start=(ci == 0), stop=(ci == 1),
                    )
                t = spool.tile([128, TT], BF16, name=f"t_{co}")
                if co == 0:
                    nc.scalar.activation(t[:], p[:], AF.Identity,
                                         bias=bias(BI_TB2, co))
                else:
                    nc.vector.tensor_scalar_add(t[:], p[:], bias(BI_TB2, co))
                tt = spool.tile([128, TT], BF16, name=f"tt_{co}")
                nc.vector.tensor_mul(tt[:], t[:], t[:])
                t_sb.append(t)
                tt_sb.append(tt)

            # x_mask penalty rides one adim row of tt (summed into |t|^2)
            penb = spool.tile([1, TT], BF16, name="penb")
            nc.vector.tensor_copy(penb[:], pp[0:1, PKP_MASK:PKP_MASK + TT])
            nc.vector.tensor_add(tt_sb[0][0:1, :], tt_sb[0][0:1, :], penb[:])

            # ---------------- d2 open: |t|^2 via ones^T @ tt ------------
            d2t = [pd2p.tile([128, 2 * TT], F32, name=f"d2_{i}", tag="d2")
                   for i in range(2)]

            def d2ap(c, h=None):
                st, hh = CHUNKS[c]
                if h is None:
                    h = 128          # ones-MMs fill all partitions
                col = (c % 2) * TT
                return d2t[c // 2][0:h, col:col + TT]

            # NOTE: start=True clears the whole PSUM *bank* (has_written is
            # per element, the clear is not) -> exactly one start per tile.
            for c in range(4):
                for ci in range(2):
                    nc.tensor.matmul(d2ap(c), ones[:], tt_sb[ci][:],
                                     start=(c % 2 == 0 and ci == 0),
                                     stop=False,
                                     skip_group_check=True)

            # ---------------- feats conv2 -------------------------------
            f2_sb = []
            for co in range(2):
                p = pconv.tile([128, HALF], F32, name=f"pf2_{co}", tag="convp",
                               padded_shape=[128, TFW])
                n = 0
                for ci in range(2):
                    for k in range(3):
                        nc.tensor.matmul(
                            p[:],
                            a2[:, 768 * ci + 256 * k + 128 * co:
                               768 * ci + 256 * k + 128 * (co + 1)],
                            f1_sb[ci][:, k:k + HALF],
                            start=(n == 0), stop=(n == 5),
                        )
                        n += 1
                f2 = spool.tile([128, HALF], BF16, name=f"f2_{co}")
                if co == 0:
                    nc.scalar.activation(f2[:], p[:], AF.Relu,
                                         bias=bias(BI_FB2, co))
                else:
                    nc.vector.tensor_scalar(f2[:], p[:], bias(BI_FB2, co), 0.0,
                                            op0=AL.add, op1=AL.max)
                f2_sb.append(f2)

            # ---------------- feats conv3: fneg2 = -2f, ffn = 4f^2 ------
            fneg2_sb, ffn_sb = [], []
            for co in range(2):
                p = pconv.tile([128, HALF], F32, name=f"pf3_{co}", tag="convp",
                               padded_shape=[128, TFW])
                for ci in range(2):
                    nc.tensor.matmul(
                        p[:],
                        a3[:, PKA3_FW3 + 256 * ci + 128 * co:
                           PKA3_FW3 + 256 * ci + 128 * (co + 1)],
                        f2_sb[ci][:],
                        start=(ci == 0), stop=(ci == 1),
                    )
                fneg2 = spool.tile([128, HALF], BF16, name=f"fneg2_{co}")
                ffn = spool.tile([128, HALF], BF16, name=f"ffn_{co}")
                if co == 0:
                    nc.scalar.activation(fneg2[:], p[:], AF.Identity,
                                         scale=-2.0, bias=bias(BI_FB3N, co))
                    nc.vector.tensor_mul(ffn[:], fneg2[:], fneg2[:])
                else:
                    nc.vector.tensor_scalar(fneg2[:], p[:], bias(BI_FB3P, co),
                                            -2.0, op0=AL.add, op1=AL.mult)
                    nc.scalar.activation(ffn[:], p[:], AF.Square,
                                         scale=-2.0, bias=bias(BI_FB3N, co))
                fneg2_sb.append(fneg2)
                ffn_sb.append(ffn)

            # |f|^2 row: ones-column matmul over ffn, scaled 0.25 on evac
            pr = psm.tile([1, HALF], F32, name="pr", tag="sm")
            for ci in range(2):
                nc.tensor.matmul(pr[:], ones[:, 0:1], ffn_sb[ci][:],
                                 start=(ci == 0), stop=(ci == 1))
            fsq = spool.tile([1, HALF], FP16, name="fsq")
            nc.vector.tensor_scalar_mul(fsq[:], pr[:], 0.25)

            # ---------------- d2 close: -2 f.t + |f|^2 ------------------
            for c, (st, h) in enumerate(CHUNKS):
                ap = d2ap(c, h)
                for ci in range(2):
                    nc.tensor.matmul(ap, fneg2_sb[ci][:, st:st + h],
                                     t_sb[ci][:],
                                     start=False, stop=False,
                                     skip_group_check=True)
                nc.tensor.matmul(ap, fsq[:, st:st + h], onesr[:],
                                 start=False, stop=(c % 2 == 1),
                                 skip_group_check=True)

            # ---------------- epilogue (per chunk, pipelined) -----------
            u = spool.tile([128, 4 * TT], FP16, name="u")
            s = spool.tile([128, 4 * TT], FP16, name="s")
            sp = spool.tile([128, 4 * TT], FP16, name="sp")
            e_scr = spool.tile([128, TT], BF16, name="e_scr")
            z = spool.tile([128, 4], F32, name="z")
            lnz = spool.tile([128, 4], F32, name="lnz")
            o = spool.tile([128, 4 * TT], FP16, name="o")

            for c, (st, h) in enumerate(CHUNKS):
                cs = slice(c * TT, (c + 1) * TT)
                uc, sc = u[0:h, cs], s[0:h, cs]
                nc.scalar.activation(uc, d2ap(c, h), AF.Ln)
                nc.scalar.activation(sc, uc, AF.Exp, scale=0.5)
                nc.vector.tensor_sub(sp[0:h, cs], pp[0:h, cs], sc)
                nc.scalar.activation(e_scr[0:h, :], sc, AF.Exp, scale=-1.0,
                                     accum_out=z[0:h, c:c + 1])
                nc.scalar.activation(lnz[0:h, c:c + 1], z[0:h, c:c + 1],
                                     AF.Ln)
                nc.vector.tensor_scalar_sub(o[0:h, cs], sp[0:h, cs],
                                            lnz[0:h, c:c + 1])
                eng = nc.sync if c % 2 == 0 else nc.gpsimd
                eng.dma_start(d_out.ap()[st:st + h, :], o[0:h, cs])

    _split_multiwait(nc)
    _nc_cache[with_biases] = nc
    return nc


# ------------------------------------------------------------------ host glue
def _bf16(a):
    return np.asarray(a, np.float32).astype(ml_dtypes.bfloat16)


def _h2(a):
    """(256, X) -> (128, 2X): ci chunk c at columns [c*X, (c+1)*X)."""
    return np.concatenate([a[:128], a[128:]], axis=1)


def _prep_shared(t_w1, t_b1, t_w2, t_b2, f_w1, f_b1, f_w2, f_b2, f_w3, f_b3):
    tw1h = np.asarray(t_w1, np.float32).transpose(1, 2, 0).reshape(ADIM, 3 * ADIM)
    tw2h = np.asarray(t_w2, np.float32)[:, :, 0].T
    fw1h = np.asarray(f_w1, np.float32).transpose(1, 2, 0).reshape(ODIM, 3 * ADIM)
    fw2h = np.asarray(f_w2, np.float32).transpose(1, 2, 0).reshape(ADIM, 3 * ADIM)
    fw3h = np.asarray(f_w3, np.float32)[:, :, 0].T

    pk_a2 = np.ascontiguousarray(_bf16(_h2(fw2h)))
    pk_a3 = np.empty((128, PKA3_W), ml_dtypes.bfloat16)
    pk_a3[:, PKA3_TW2:PKA3_TW2 + 2 * 256] = _bf16(_h2(tw2h))
    pk_a3[:, PKA3_FW3:PKA3_FW3 + 2 * 256] = _bf16(_h2(fw3h))

    pk_b = np.zeros((256, PKB_W // 2), np.float32)
    for j, v in enumerate([f_b1, f_b2, t_b1, t_b2, f_b3,
                           -2.0 * np.asarray(f_b3)]):
        pk_b[:, j] = np.asarray(v, np.float32)

    return {
        "pk_a2": pk_a2,
        "pk_a3": pk_a3,
        "pk_b": np.ascontiguousarray(_h2(pk_b)),   # (128, 12)
        "tw1h2": _bf16(_h2(tw1h)),
        "fw1h": _bf16(fw1h),
    }


def _prep_core_inputs(c, text, feats, x_masks, shared):
    b, h = divmod(c, 2)
    s = h * HALF

    pk_f = np.zeros((ODIM, PKF_W), ml_dtypes.bfloat16)
    lo, hi = max(0, s - 2), min(T_FEATS, s + TFW)
    pk_f[:, lo - (s - 2):hi - (s - 2)] = _bf16(feats[b, lo:hi].T)
    pk_f[:, PKF_FW1:] = shared["fw1h"]

    textT = np.zeros((ADIM, TTP), np.float32)
    textT[:, 1:1 + TT] = text[b].T
    pk_a1 = np.empty((128, PKA1_W), ml_dtypes.bfloat16)
    pk_a1[:, PKA1_TEXT:PKA1_TEXT + 2 * TTP] = _bf16(_h2(textT))
    pk_a1[:, PKA1_TW1:PKA1_TW1 + 2 * 768] = shared["tw1h2"]
    pk_a1[:, PKA1_MASK] = 0.0 if s - 1 < 0 else 1.0
    pk_a1[:, PKA1_MASK + 1] = 0.0 if s + HALF >= T_FEATS else 1.0

    prior = _beta_binomial_prior()[s:s + HALF]               # (400, 160)
    pk_p = np.zeros((128, PKP_W), np.float16)
    for ci, (st, hh) in enumerate(CHUNKS):
        pk_p[0:hh, ci * TT:(ci + 1) * TT] = prior[st:st + hh].astype(np.float16)
    pk_p[0, PKP_MASK:PKP_MASK + TT] = (
        MASK_PENALTY * x_masks[b].astype(np.float16))
    return {
        "pk_b": shared["pk_b"],
        "pk_fw": pk_f,
        "pk_a1": pk_a1,
        "pk_a2": shared["pk_a2"],
        "pk_a3": shared["pk_a3"],
        "pk_p": pk_p,
    }


def kernel(text, feats, text_lengths, feats_lengths, x_masks,
           t_w1, t_b1, t_w2, t_b2, f_w1, f_b1, f_w2, f_b2, f_w3, f_b3):
    text = np.asarray(text, np.float32)
    feats = np.asarray(feats, np.float32)
    x_masks = np.asarray(x_masks)

    shared = _prep_shared(t_w1, t_b1, t_w2, t_b2,
                          f_w1, f_b1, f_w2, f_b2, f_w3, f_b3)
    with_biases = bool(shared["pk_b"].any())
    nc = _build_program(with_biases)
    in_maps = []
    for c in range(N_CORES):
        m = _prep_core_inputs(c, text, feats, x_masks, shared)
        if not with_biases:
            m.pop("pk_b")
        in_maps.append(m)
    res = None
    last_exc = None
    for _attempt in range(3):
        try:
            res = run_bass_kernel_spmd(nc, in_maps,
                                       core_ids=list(range(N_CORES)))
            break
        except Exception as e:   # transient NRT exec-unit flake on cold NEFFs
            last_exc = e
    if res is None:
        raise last_exc

    out = np.empty((B, T_FEATS, T_TEXT), np.float32)
    for c in range(N_CORES):
        b, h = divmod(c, 2)
        out[b, h * HALF:(h + 1) * HALF, :] = (
            res.results[c]["out"].astype(np.float32))
    return out


# revision 11
# speedup vs baseline: 1.0406x; 1.0406x over previous
"""Trainium2 Bass kernel for nn_AlignmentModule (conv stems + L2 score +
log-softmax + beta-binomial prior).

Sharding: 8 cores = 4 batches x 2 T_feats halves (400 rows each). Each core
computes the text conv stem for its batch, its half of the feats conv stem,
and the (400, 160) score block in FEATS-MAJOR layout (feats positions on
psum partitions in 128/128/128/16 chunks, text on the free dim).

Feats-major design (vs the text-major ~31us baseline):
 - d2 = |f|^2 - 2 f.t + |t|^2 assembled per chunk in PSUM:
   ones^T @ t^2 matmuls give |t_j|^2 (start), fneg2^T @ t give -2 f.t,
   and a K=1 rank-1 (|f|^2 row x ones) closes the group. |f|^2 row comes
   from one ones-column matmul over ffn = (-2f)^2 (scaled 0.25 on evac).
 - softmax over text = the FREE dim: the ACT Exp pass's accum_out port
   yields Z per feats row for free; out = (prior - s) - lnZ is two cheap
   DVE passes. No transposes anywhere, no PE work in the softmax, and the
   output leaves in the reference (feats, text) orientation as fp16.
 - PE warm-up: 8 N=512 dummy matmuls issued immediately (~3.4us at the
   cold 1.2 GHz clock) so the free-running HAM window unthrottles the PE
   to 2.4 GHz right as the input DMAs land and the real matmuls start.
 - input DMA descriptors spread across 4 engine queues (sync/scalar/
   gpsimd/vector) instead of serializing on one.
 - all ACT functions used (exp/ln/relu/identity/square) live in the
   natural_log_exp_and_others table set -> one table load, warmed at t=0.

Self-contained: hardcodes all shapes; reads nothing from disk.
"""

import math
import os
import subprocess
import sys

import numpy as np
import ml_dtypes

import concourse.bass as bass
import concourse.mybir as mybir
import concourse.tile as tile
from concourse.bass_utils import run_bass_kernel_spmd

B, T_TEXT, T_FEATS = 4, 160, 800
ADIM, ODIM = 256, 80
N_CORES = 8
HALF = T_FEATS // 2          # 400 feats rows per core
TT = T_TEXT                  # 160
TFW = HALF + 2               # 402: f1 window  [s-1, s+401)
TFIN = HALF + 4              # 404: feats input window [s-2, s+402)
TTP = TT + 2                 # 162: text input window (zero halo cols)
F32 = mybir.dt.float32
BF16 = mybir.dt.bfloat16
FP16 = mybir.dt.float16
MASK_PENALTY = 6.0e4         # d2 += 6e4 -> s ~ 245 -> exp(-s) == 0 in f32
CHUNKS = [(0, 128), (128, 128), (256, 128), (384, 16)]

# ---- input pack layouts -------------------------------------------------
# pk_fw (80, PKF_W) bf16: featsT window | fw1
PKF_FEATS = 0
PKF_FW1 = TFIN
PKF_W = TFIN + 3 * 256

# pk_a1 (128, PKA1_W) bf16: textT (2ci x TTP) | tw1 (2ci x 768) | f1 halo masks
PKA1_TEXT = 0
PKA1_TW1 = 2 * TTP
PKA1_MASK = PKA1_TW1 + 2 * 768
PKA1_W = PKA1_MASK + 2

# pk_a2 (128, 1536) bf16: fw2
PKA2_W = 2 * 768

# pk_a3 (128, 1024) bf16: tw2 (2ci x 256) | fw3 (2ci x 256)
PKA3_TW2 = 0
PKA3_FW3 = 2 * 256
PKA3_W = 4 * 256

# pk_p (128, PKP_W) fp16: prior chunks (4 x 160, chunk c rows at cols
# [160c,160c+160)) | mask penalty row (row 0 of cols 640:800)
PKP_PRIOR = 0
PKP_MASK = 4 * TT
PKP_W = 5 * TT

# pk_b (128, 12) f32 bias columns (2 co-halves each)
BI_FB1 = 0    # f_b1
BI_FB2 = 2    # f_b2
BI_TB1 = 4    # t_b1
BI_TB2 = 6    # t_b2
BI_FB3P = 8   # +f_b3
BI_FB3N = 10  # -2*f_b3
PKB_W = 12

_nc_cache = {}
_prior_cache = None


# ---------------------------------------------------------------- host math
def _prior_f64():
    """f64 fallback replica of reference.beta_binomial_prior."""
    try:
        from scipy.special import gammaln as _gl
    except Exception:
        _gl = np.vectorize(math.lgamma)
    T, N = float(T_FEATS), float(T_TEXT)
    a = np.arange(1, T_FEATS + 1, dtype=np.float64)
    b = T - a + 1.0
    k = np.arange(T_TEXT, dtype=np.float64)[:, None]

    def betaln(x, y):
        return _gl(x) + _gl(y) - _gl(x + y)

    logp = (
        _gl(N + 1.0) - _gl(k + 1.0) - _gl(N - k + 1.0)
        + betaln(k + a, N - k + b) - betaln(a, b)
    )
    return np.asarray(logp.T, dtype=np.float32)


_NIX_SITE = ("/nix/store/z022hj2nvbm3nwdizlisq4ylc0y7rd6q-python3-3.13.14-env"
             "/lib/python3.13/site-packages")

_PRIOR_SRC = """
import os
os.environ["JAX_PLATFORMS"] = "cpu"
import numpy as np
import jax.numpy as jnp
from jax.scipy.special import gammaln

T, N = {T}, {N}
a = 1.0 * jnp.arange(1, T + 1, dtype=jnp.float32)
b = 1.0 * (T - a + 1.0)
k = jnp.arange(N, dtype=jnp.float32)[:, None]
Nf = jnp.float32(N)

def betaln(x, y):
    return gammaln(x) + gammaln(y) - gammaln(x + y)

logp = (gammaln(Nf + 1.0) - gammaln(k + 1.0) - gammaln(Nf - k + 1.0)
        + betaln(k + a, Nf - k + b) - betaln(a, b))
np.save({out!r}, np.asarray(logp.T, dtype=np.float32))
"""


def _beta_binomial_prior():
    """beta_binomial_prior(T_FEATS, T_TEXT) matching the reference's jax
    f32 computation, via a jax-CPU subprocess (python -S skips the
    sitecustomize that would force the axon/neuron backend). Falls back
    to a scipy f64 replica (abs diff ~1e-3, harmless at the 2e-2 gate)."""
    global _prior_cache
    if _prior_cache is not None:
        return _prior_cache
    cache = f"/tmp/_bbprior_{T_FEATS}x{T_TEXT}.npy"
    if not os.path.exists(cache):
        src = _PRIOR_SRC.format(T=T_FEATS, N=T_TEXT, out=cache)
        for _attempt in range(2):
            try:
                env = dict(os.environ)
                env["JAX_PLATFORMS"] = "cpu"
                env["TRN_TERMINAL_POOL_IPS"] = ""
                env["PYTHONPATH"] = _NIX_SITE
                r = subprocess.run([sys.executable, "-S", "-c", src],
                                   capture_output=True, timeout=120, env=env)
                if r.returncode == 0 and os.path.exists(cache):
                    break
            except Exception:
                pass
    if os.path.exists(cache):
        _prior_cache = np.load(cache).astype(np.float32)
    else:
        _prior_cache = _prior_f64()
    return _prior_cache


# ------------------------------------------------------------- BIR patching
def _split_multiwait(nc):
    """This container's walrus accepts at most one sync wait per
    instruction; move extras onto single-wait NOPs just before."""
    for f in nc.m.functions:
        for bb in f.blocks:
            changed = False
            out = []
            for inst in bb.instructions:
                si = inst.sync_info
                if si is not None and len(si.on_wait) > 1:
                    waits = list(si.on_wait)
                    for j, w in enumerate(waits[:-1]):
                        nop = mybir.InstNoOp(name=f"{inst.name}sw{j}")
                        nop.name = f"{inst.name}sw{j}"
                        nop.engine = inst.engine
                        nop.sync_info = mybir.SyncInfo(on_wait=[w], on_update=[])
                        out.append(nop)
                    inst.sync_info = mybir.SyncInfo(
                        on_wait=[waits[-1]], on_update=list(si.on_update)
                    )
                    changed = True
                out.append(inst)
            if changed:
                bb.instructions = out


# ------------------------------------------------------------ device program
def _build_program(with_biases=False):
    if with_biases in _nc_cache:
        return _nc_cache[with_biases]

    nc = bass.Bass("TRN2", target_bir_lowering=False, debug=False,
                   num_devices=N_CORES, enable_asserts=False)
    AF = mybir.ActivationFunctionType
    AL = mybir.AluOpType

    if with_biases:
        d_pb = nc.dram_tensor("pk_b", [128, PKB_W], F32, kind="ExternalInput")
    d_pf = nc.dram_tensor("pk_fw", [ODIM, PKF_W], BF16, kind="ExternalInput")
    d_a1 = nc.dram_tensor("pk_a1", [128, PKA1_W], BF16, kind="ExternalInput")
    d_a2 = nc.dram_tensor("pk_a2", [128, PKA2_W], BF16, kind="ExternalInput")
    d_a3 = nc.dram_tensor("pk_a3", [128, PKA3_W], BF16, kind="ExternalInput")
    d_pp = nc.dram_tensor("pk_p", [128, PKP_W], FP16, kind="ExternalInput")
    d_out = nc.dram_tensor("out", [HALF, TT], FP16, kind="ExternalOutput")

    with tile.TileContext(nc) as tc:
        with (
            tc.tile_pool(name="dpool", bufs=1) as dpool,
            tc.tile_pool(name="spool", bufs=1) as spool,
            tc.tile_pool(name="pwp", bufs=1, space="PSUM") as pwp,
            tc.tile_pool(name="pconv", bufs=3, space="PSUM") as pconv,
            tc.tile_pool(name="pd2", bufs=2, space="PSUM") as pd2p,
            tc.tile_pool(name="psm", bufs=1, space="PSUM") as psm,
        ):
            # ---------------- input DMAs on 3 queues --------------------
            # pf rides the scalar queue (observed to win DMA arbitration)
            # since f1 needs it first; a1 on sync.
            pf = dpool.tile([ODIM, PKF_W], BF16, name="pf")
            nc.scalar.dma_start(pf[0:16, :], d_pf.ap()[0:16, :])
            nc.scalar.dma_start(pf[16:ODIM, :], d_pf.ap()[16:ODIM, :])
            a1 = dpool.tile([128, PKA1_W], BF16, name="a1")
            nc.sync.dma_start(a1[:], d_a1.ap())
            a3 = dpool.tile([128, PKA3_W], BF16, name="a3")
            nc.gpsimd.dma_start(a3[:], d_a3.ap())
            a2 = dpool.tile([128, PKA2_W], BF16, name="a2")
            nc.gpsimd.dma_start(a2[:], d_a2.ap())
            pp = dpool.tile([128, PKP_W], FP16, name="pp")
            nc.gpsimd.dma_start(pp[:], d_pp.ap())

            # warm-up source memset first on DVE
            wscr = spool.tile([128, 512], BF16, name="wscr")
            nc.vector.memset(wscr[:], 1.0)

            pb = dpool.tile([128, PKB_W], F32, name="pb")
            if with_biases:
                nc.sync.dma_start(pb[:], d_pb.ap())
            else:
                nc.gpsimd.memset(pb[:], 0.0)

            def bias(base, j):
                return pb[:, base + j: base + j + 1]

            # ---------------- on-chip constants + warms -----------------
            ones = spool.tile([128, 128], BF16, name="ones")
            nc.gpsimd.memset(ones[:], 1.0)
            onesr = spool.tile([1, TT], FP16, name="onesr")
            nc.gpsimd.memset(onesr[:], 1.0)

            # warm the ACT table set (natural_log_exp_and_others covers
            # exp/ln/relu/identity/square) during the input DMAs
            scr = spool.tile([1, 4], F32, name="scr")
            nc.scalar.activation(scr[0:1, 1:2], scr[0:1, 3:4], AF.Exp)
            nc.scalar.activation(scr[0:1, 2:3], scr[0:1, 3:4], AF.Ln)

            # PE HAM warm-up: ~3.4us of dense cold matmuls so the HAM
            # SHORT window unthrottles the PE right as real work starts
            pwarm = pwp.tile([128, 512], F32, name="pwarm", tag="warm")
            for _ in range(8):
                nc.tensor.matmul(pwarm[:], wscr[:, 0:128], wscr[:],
                                 start=True, stop=True,
                                 skip_group_check=True)

            # ---------------- feats conv1 -------------------------------
            f1_sb = []
            for co in range(2):
                p = pconv.tile([128, TFW], F32, name=f"pf1_{co}", tag="convp")
                for k in range(3):
                    nc.tensor.matmul(
                        p[:],
                        pf[:, PKF_FW1 + 256 * k + 128 * co:
                           PKF_FW1 + 256 * k + 128 * (co + 1)],
                        pf[:, k:k + TFW],
                        start=(k == 0), stop=(k == 2),
                    )
                f1 = spool.tile([128, TFW], BF16, name=f"f1_{co}")
                if co == 0:
                    nc.scalar.activation(f1[:], p[:], AF.Relu,
                                         bias=bias(BI_FB1, co))
                else:
                    nc.vector.tensor_scalar(f1[:], p[:], bias(BI_FB1, co), 0.0,
                                            op0=AL.add, op1=AL.max)
                # zero the halo column the reference conv padding zeroes
                nc.vector.tensor_mul(f1[:, 0:1], f1[:, 0:1],
                                     a1[:, PKA1_MASK:PKA1_MASK + 1])
                nc.vector.tensor_mul(f1[:, TFW - 1:TFW], f1[:, TFW - 1:TFW],
                                     a1[:, PKA1_MASK + 1:PKA1_MASK + 2])
                f1_sb.append(f1)

            # ---------------- text conv1 --------------------------------
            t1_sb = []
            for co in range(2):
                p = pconv.tile([128, TT], F32, name=f"pt1_{co}", tag="convp",
                               padded_shape=[128, TFW])
                n = 0
                for ci in range(2):
                    for k in range(3):
                        nc.tensor.matmul(
                            p[:],
                            a1[:, PKA1_TW1 + 768 * ci + 256 * k + 128 * co:
                               PKA1_TW1 + 768 * ci + 256 * k + 128 * (co + 1)],
                            a1[:, TTP * ci + k: TTP * ci + k + TT],
                            start=(n == 0), stop=(n == 5),
                        )
                        n += 1
                t1 = spool.tile([128, TT], BF16, name=f"t1_{co}")
                if co == 0:
                    nc.scalar.activation(t1[:], p[:], AF.Relu,
                                         bias=bias(BI_TB1, co))
                else:
                    nc.vector.tensor_scalar(t1[:], p[:], bias(BI_TB1, co), 0.0,
                                            op0=AL.add, op1=AL.max)
                t1_sb.append(t1)

            # ---------------- text conv2: t, tt = t^2 -------------------
            t_sb, tt_sb = [], []
            for co in range(2):
                p = pconv.tile([128, TT], F32, name=f"pt2_{co}", tag="convp",
                               padded_shape=[128, TFW])
                for ci in range(2):
                    nc.tensor.matmul(
                        p[:],
                        a3[:, PKA3_TW2 + 256 * ci + 128 * co:
                           PKA3_TW2 + 256 * ci + 128 * (co + 1)],
                        t1_sb[ci][:],
                        start=(ci == 0), stop=(ci == 1),
                    )
                t = spool.tile([128, TT], BF16, name=f"t_{co}")
                if co == 0:
                    nc.scalar.activation(t[:], p[:], AF.Identity,
                                         bias=bias(BI_TB2, co))
                else:
                    nc.vector.tensor_scalar_add(t[:], p[:], bias(BI_TB2, co))
                tt = spool.tile([128, TT], BF16, name=f"tt_{co}")
                nc.vector.tensor_mul(tt[:], t[:], t[:])
                t_sb.append(t)
                tt_sb.append(tt)

            # x_mask penalty rides one adim row of tt (summed into |t|^2)
            penb = spool.tile([1, TT], BF16, name="penb")
            nc.vector.tensor_copy(penb[:], pp[0:1, PKP_MASK:PKP_MASK + TT])
            nc.vector.tensor_add(tt_sb[0][0:1, :], tt_sb[0][0:1, :], penb[:])

            # ---------------- d2 open: |t|^2 via ones^T @ tt ------------
            d2t = [pd2p.tile([128, 2 * TT], F32, name=f"d2_{i}", tag="d2")
                   for i in range(2)]

            def d2ap(c, h=None):
                st, hh = CHUNKS[c]
                if h is None:
                    h = 128          # ones-MMs fill all partitions
                col = (c % 2) * TT
                return d2t[c // 2][0:h, col:col + TT]

            # NOTE: start=True clears the whole PSUM *bank* (has_written is
            # per element, the clear is not) -> exactly one start per tile.
            for c in range(4):
                for ci in range(2):
                    nc.tensor.matmul(d2ap(c), ones[:], tt_sb[ci][:],
                                     start=(c % 2 == 0 and ci == 0),
                                     stop=False,
                                     skip_group_check=True)

            # ---------------- feats conv2 -------------------------------
            f2_sb = []
            for co in range(2):
                p = pconv.tile([128, HALF], F32, name=f"pf2_{co}", tag="convp",
                               padded_shape=[128, TFW])
                n = 0
                for ci in range(2):
                    for k in range(3):
                        nc.tensor.matmul(
                            p[:],
                            a2[:, 768 * ci + 256 * k + 128 * co:
                               768 * ci + 256 * k + 128 * (co + 1)],
                            f1_sb[ci][:, k:k + HALF],
                            start=(n == 0), stop=(n == 5),
                        )
                        n += 1
                f2 = spool.tile([128, HALF], BF16, name=f"f2_{co}")
                if co == 0:
                    nc.scalar.activation(f2[:], p[:], AF.Relu,
                                         bias=bias(BI_FB2, co))
                else:
                    nc.vector.tensor_scalar(f2[:], p[:], bias(BI_FB2, co), 0.0,
                                            op0=AL.add, op1=AL.max)
                f2_sb.append(f2)

            # ---------------- feats conv3: fneg2 = -2f, ffn = 4f^2 ------
            fneg2_sb, ffn_sb = [], []
            for co in range(2):
                p = pconv.tile([128, HALF], F32, name=f"pf3_{co}", tag="convp",
                               padded_shape=[128, TFW])
                for ci in range(2):
                    nc.tensor.matmul(
                        p[:],
                        a3[:, PKA3_FW3 + 256 * ci + 128 * co:
                           PKA3_FW3 + 256 * ci + 128 * (co + 1)],
                        f2_sb[ci][:],
                        start=(ci == 0), stop=(ci == 1),
                    )
                fneg2 = spool.tile([128, HALF], BF16, name=f"fneg2_{co}")
                ffn = spool.tile([128, HALF], BF16, name=f"ffn_{co}")
                # ffn = 4(f+b)^2; the 1/4 is folded into the fsq evac
                if co == 0:
                    nc.scalar.activation(fneg2[:], p[:], AF.Identity,
                                         scale=-2.0, bias=bias(BI_FB3N, co))
                    nc.vector.tensor_mul(ffn[:], fneg2[:], fneg2[:])
                else:
                    nc.vector.tensor_scalar(fneg2[:], p[:], bias(BI_FB3P, co),
                                            -2.0, op0=AL.add, op1=AL.mult)
                    nc.scalar.activation(ffn[:], p[:], AF.Square,
                                         scale=-2.0, bias=bias(BI_FB3N, co))
                fneg2_sb.append(fneg2)
                ffn_sb.append(ffn)

            # |f|^2 row: ones-column matmul over ffn, scaled 0.25 on evac
            pr = psm.tile([1, HALF], F32, name="pr", tag="sm")
            for ci in range(2):
                nc.tensor.matmul(pr[:], ones[:, 0:1], ffn_sb[ci][:],
                                 start=(ci == 0), stop=(ci == 1))
            fsq = spool.tile([1, HALF], FP16, name="fsq")
            nc.vector.tensor_scalar_mul(fsq[:], pr[:], 0.25)

            # ---------------- d2 close: -2 f.t + |f|^2 ------------------
            for c, (st, h) in enumerate(CHUNKS):
                ap = d2ap(c, h)
                for ci in range(2):
                    nc.tensor.matmul(ap, fneg2_sb[ci][:, st:st + h],
                                     t_sb[ci][:],
                                     start=False, stop=False,
                                     skip_group_check=True)
                nc.tensor.matmul(ap, fsq[:, st:st + h], onesr[:],
                                 start=False, stop=(c % 2 == 1),
                                 skip_group_check=True)

            # ---------------- epilogue (wide passes per tile) -----------
            # Chunk 3's unused partitions 16:128 carry |t|^2-only psum
            # values: positive, so Ln/Exp stay finite; never DMA'd.
            u = spool.tile([128, 4 * TT], FP16, name="u")
            s = spool.tile([128, 4 * TT], FP16, name="s")
            sp = spool.tile([128, 4 * TT], FP16, name="sp")
            e_sb = [spool.tile([128, 2, TT], BF16, name=f"e_{i}")
                    for i in range(2)]
            z = spool.tile([128, 4], F32, name="z")
            lnz = spool.tile([128, 4], F32, name="lnz")
            o = spool.tile([128, 4 * TT], FP16, name="o")

            for ti in range(2):
                c2 = slice(2 * ti * TT, 2 * (ti + 1) * TT)
                zc = slice(2 * ti, 2 * ti + 2)
                nc.scalar.activation(u[:, c2], d2t[ti][:], AF.Ln)
                nc.scalar.activation(s[:, c2], u[:, c2], AF.Exp, scale=0.5)
                nc.vector.tensor_sub(sp[:, c2], pp[:, c2], s[:, c2])
                nc.scalar.activation(e_sb[ti][:], s[:, c2], AF.Exp,
                                     scale=-1.0)
                nc.vector.tensor_reduce(z[:, zc], e_sb[ti][:],
                                        mybir.AxisListType.X, AL.add)
                nc.scalar.activation(lnz[:, zc], z[:, zc], AF.Ln)
                for j in range(2):
                    c = 2 * ti + j
                    st, h = CHUNKS[c]
                    cs = slice(c * TT, (c + 1) * TT)
                    nc.vector.tensor_scalar_sub(o[0:h, cs], sp[0:h, cs],
                                                lnz[0:h, c:c + 1])
                    eng = nc.sync if c % 2 == 0 else nc.gpsimd
                    eng.dma_start(d_out.ap()[st:st + h, :], o[0:h, cs])

    _split_multiwait(nc)
    _nc_cache[with_biases] = nc
    return nc


# ------------------------------------------------------------------ host glue
def _bf16(a):
    return np.asarray(a, np.float32).astype(ml_dtypes.bfloat16)


def _h2(a):
    """(256, X) -> (128, 2X): ci chunk c at columns [c*X, (c+1)*X)."""
    return np.concatenate([a[:128], a[128:]], axis=1)


def _prep_shared(t_w1, t_b1, t_w2, t_b2, f_w1, f_b1, f_w2, f_b2, f_w3, f_b3):
    tw1h = np.asarray(t_w1, np.float32).transpose(1, 2, 0).reshape(ADIM, 3 * ADIM)
    tw2h = np.asarray(t_w2, np.float32)[:, :, 0].T
    fw1h = np.asarray(f_w1, np.float32).transpose(1, 2, 0).reshape(ODIM, 3 * ADIM)
    fw2h = np.asarray(f_w2, np.float32).transpose(1, 2, 0).reshape(ADIM, 3 * ADIM)
    fw3h = np.asarray(f_w3, np.float32)[:, :, 0].T

    pk_a2 = np.ascontiguousarray(_bf16(_h2(fw2h)))
    pk_a3 = np.empty((128, PKA3_W), ml_dtypes.bfloat16)
    pk_a3[:, PKA3_TW2:PKA3_TW2 + 2 * 256] = _bf16(_h2(tw2h))
    pk_a3[:, PKA3_FW3:PKA3_FW3 + 2 * 256] = _bf16(_h2(fw3h))

    pk_b = np.zeros((256, PKB_W // 2), np.float32)
    for j, v in enumerate([f_b1, f_b2, t_b1, t_b2, f_b3,
                           -2.0 * np.asarray(f_b3)]):
        pk_b[:, j] = np.asarray(v, np.float32)

    return {
        "pk_a2": pk_a2,
        "pk_a3": pk_a3,
        "pk_b": np.ascontiguousarray(_h2(pk_b)),   # (128, 12)
        "tw1h2": _bf16(_h2(tw1h)),
        "fw1h": _bf16(fw1h),
    }


def _prep_core_inputs(c, text, feats, x_masks, shared):
    b, h = divmod(c, 2)
    s = h * HALF

    pk_f = np.zeros((ODIM, PKF_W), ml_dtypes.bfloat16)
    lo, hi = max(0, s - 2), min(T_FEATS, s + TFW)
    pk_f[:, lo - (s - 2):hi - (s - 2)] = _bf16(feats[b, lo:hi].T)
    pk_f[:, PKF_FW1:] = shared["fw1h"]

    textT = np.zeros((ADIM, TTP), np.float32)
    textT[:, 1:1 + TT] = text[b].T
    pk_a1 = np.empty((128, PKA1_W), ml_dtypes.bfloat16)
    pk_a1[:, PKA1_TEXT:PKA1_TEXT + 2 * TTP] = _bf16(_h2(textT))
    pk_a1[:, PKA1_TW1:PKA1_TW1 + 2 * 768] = shared["tw1h2"]
    pk_a1[:, PKA1_MASK] = 0.0 if s - 1 < 0 else 1.0
    pk_a1[:, PKA1_MASK + 1] = 0.0 if s + HALF >= T_FEATS else 1.0

    prior = _beta_binomial_prior()[s:s + HALF]               # (400, 160)
    pk_p = np.zeros((128, PKP_W), np.float16)
    for ci, (st, hh) in enumerate(CHUNKS):
        pk_p[0:hh, ci * TT:(ci + 1) * TT] = prior[st:st + hh].astype(np.float16)
    pk_p[0, PKP_MASK:PKP_MASK + TT] = (
        MASK_PENALTY * x_masks[b].astype(np.float16))
    return {
        "pk_b": shared["pk_b"],
        "pk_fw": pk_f,
        "pk_a1": pk_a1,
        "pk_a2": shared["pk_a2"],
        "pk_a3": shared["pk_a3"],
        "pk_p": pk_p,
    }


def kernel(text, feats, text_lengths, feats_lengths, x_masks,
           t_w1, t_b1, t_w2, t_b2, f_w1, f_b1, f_w2, f_b2, f_w3, f_b3):
    text = np.asarray(text, np.float32)
    feats = np.asarray(feats, np.float32)
    x_masks = np.asarray(x_masks)

    shared = _prep_shared(t_w1, t_b1, t_w2, t_b2,
                          f_w1, f_b1, f_w2, f_b2, f_w3, f_b3)
    with_biases = bool(shared["pk_b"].any())
    nc = _build_program(with_biases)
    in_maps = []
    for c in range(N_CORES):
        m = _prep_core_inputs(c, text, feats, x_masks, shared)
        if not with_biases:
            m.pop("pk_b")
        in_maps.append(m)
    res = None
    last_exc = None
    for _attempt in range(3):
        try:
            res = run_bass_kernel_spmd(nc, in_maps,
                                       core_ids=list(range(N_CORES)))
            break
        except Exception as e:   # transient NRT exec-unit flake on cold NEFFs
            last_exc = e
    if res is None:
        raise last_exc

    out = np.empty((B, T_FEATS, T_TEXT), np.float32)
    for c in range(N_CORES):
        b, h = divmod(c, 2)
        out[b, h * HALF:(h + 1) * HALF, :] = (
            res.results[c]["out"].astype(np.float32))
    return out
